# revision 1
# baseline (speedup 1.0000x reference)
"""ConvLocalAttention2d (7x7 window, 4 heads, d_head=16) on 8 trn2 NeuronCores.

Sharding: one (batch, head) pair per core  (B=2 x heads=4 = 8 cores), zero
cross-core communication.

Per-core algorithm (d=16, H=W=96, K=7x7=49):
  - Queries are processed in 8x16 = 128-pixel tiles (12 x 6 = 72 tiles).
  - For each tile, the key/value halo is 14x22 = 308 pixels, split into 3
    row-chunks of <=128 keys (5*22, 5*22, 4*22).
  - C1 (scores, transposed): for each chunk, PE matmul
        scoresT[keys, 128q] = k_aug_chunk.T @ q_aug_tile
    contracting over 65 "channels": 16 data channels + 1 image-boundary
    indicator + 16 y-position one-hot (mod 16) + 32 x-position one-hot
    (mod 32).  The position channels add 0 for in-window pairs and -1e9
    for out-of-window pairs, so the 7x7 window mask and the image-boundary
    mask are folded into the matmul itself (exact: the mod-16/mod-32
    aliases never collide within a tile's +-10 / +-18 offset range).
  - exp: one ACT pass per 2 tiles, PSUM->SBUF, scale=1/sqrt(16), out bf16.
  - C2: PE matmul out_u[128q, 17] += expT_chunk.T @ vt_chunk, where vt has
    17 columns: 16 v channels + a column of ones (inside the image) whose
    output is the softmax denominator Z.
  - DVE: rz = 1/Z, out = out_u[:, :16] * rz (per-partition scalar), DMA out.
"""

import functools
import numpy as np
import ml_dtypes

B = 2
HEADS = 4
DH = 16
H = W = 96
PAD = 3
PH = PW = H + 2 * PAD  # 102
TY, TX = 8, 16           # query tile shape
GY, GX = H // TY, W // TX  # 12 x 6 tile grid
HY, HX = TY + 6, TX + 6    # halo 14 x 22
NCH = 1 + 16 + 32          # img + y-onehot(16) + x-onehot(32)
D = DH + NCH               # 65 contraction channels
NEG = -1.0e9
CHUNKS = [(0, 5), (5, 5), (10, 4)]  # (row0, nrows) of halo row chunks

BF16 = ml_dtypes.bfloat16

Y_OK = {d % 16 for d in range(-3, 4)}
X_OK = {d % 32 for d in range(-3, 4)}


@functools.lru_cache(maxsize=1)
def _mask_channels():
    """Constant (core-independent) aug channels.

    q side: [NCH, H, W]  (ones, y-onehot, x-onehot)
    k side: [NCH, PH, PW] (img indicator, y-g-maps, x-g-maps), value 0 where
    the channel does not veto the pair, NEG where it does.
    """
    qm = np.zeros((NCH, H, W), np.float32)
    qm[0] = 1.0
    yy = np.arange(H)
    xx = np.arange(W)
    for r in range(16):
        qm[1 + r, yy % 16 == r, :] = 1.0
    for r in range(32):
        qm[17 + r, :, xx % 32 == r] = 1.0

    km = np.zeros((NCH, PH, PW), np.float32)
    km[0] = NEG
    km[0, PAD:PAD + H, PAD:PAD + W] = 0.0
    py = np.arange(PH)
    px = np.arange(PW)
    for r in range(16):
        bad = np.array([((y - PAD - r) % 16) not in Y_OK for y in py])
        km[1 + r, bad, :] = NEG
    for r in range(32):
        bad = np.array([((x - PAD - r) % 32) not in X_OK for x in px])
        km[17 + r, :, bad] = NEG
    return qm, km


def _host_prep(q, k, v):
    """Full [2,64,96,96] f32 inputs -> list of 8 per-core input dicts."""
    qm, km = _mask_channels()
    in_maps = []
    for core in range(8):
        b, h = divmod(core, HEADS)
        qs = q[b, DH * h:DH * h + DH]          # [16, 96, 96]
        ks = k[b, DH * h:DH * h + DH]
        vs = v[b, DH * h:DH * h + DH]

        q_aug = np.empty((D, H, W), np.float32)
        q_aug[:DH] = qs
        q_aug[DH:] = qm
        # [D, GY, TY, GX, TX] -> [D, GY*GX, TY*TX]
        q_tiled = np.ascontiguousarray(
            q_aug.reshape(D, GY, TY, GX, TX).transpose(0, 1, 3, 2, 4)
            .reshape(D, GY * GX, TY * TX))

        k_aug = np.empty((D, PH, PW), np.float32)
        k_aug[:DH] = 0.0
        k_aug[:DH, PAD:PAD + H, PAD:PAD + W] = ks
        k_aug[DH:] = km
        # per-tile halo, flattened: [D, 72, 308]
        k_tiled = np.empty((D, GY * GX, HY * HX), np.float32)
        for ty in range(GY):
            for tx in range(GX):
                k_tiled[:, ty * GX + tx] = k_aug[
                    :, TY * ty:TY * ty + HY,
                    TX * tx:TX * tx + HX].reshape(D, HY * HX)

        vt = np.zeros((PH, PW, DH + 1), np.float32)
        vt[PAD:PAD + H, PAD:PAD + W, :DH] = np.transpose(vs, (1, 2, 0))
        vt[PAD:PAD + H, PAD:PAD + W, DH] = 1.0
        # key-within-chunk major so the whole tensor loads in one DMA and
        # stays SBUF-resident: [110, 72, 3, 17]
        vt_res = np.zeros((110, GY * GX, 3, DH + 1), np.float32)
        for ty in range(GY):
            for tx in range(GX):
                halo = vt[TY * ty:TY * ty + HY,
                          TX * tx:TX * tx + HX, :].reshape(HY * HX, DH + 1)
                for c, (r0, nr) in enumerate(CHUNKS):
                    nk = nr * HX
                    vt_res[:nk, ty * GX + tx, c, :] = halo[
                        HX * r0:HX * r0 + nk]

        in_maps.append({
            "q_tiled": q_tiled.astype(BF16),
            "k_tiled": k_tiled.astype(BF16),
            "vt_res": vt_res.astype(BF16),
        })
    return in_maps


@functools.lru_cache(maxsize=1)
def _build_program():
    from contextlib import ExitStack
    import concourse.bass as bass
    import concourse.tile as tile
    from concourse import bacc, mybir

    f32 = mybir.dt.float32
    bf16 = mybir.dt.bfloat16

    nc = bacc.Bacc("TRN2", target_bir_lowering=False, debug=False,
                   num_devices=8)
    q_d = nc.dram_tensor("q_tiled", [D, GY * GX, TY * TX], bf16,
                         kind="ExternalInput").ap()
    k_d = nc.dram_tensor("k_tiled", [D, GY * GX, HY * HX], bf16,
                         kind="ExternalInput").ap()
    vt_d = nc.dram_tensor("vt_res", [110, GY * GX, 3, DH + 1], bf16,
                          kind="ExternalInput").ap()
    # flat [query-in-tile, tile, d] order; host un-permutes
    out_d = nc.dram_tensor("out", [TY * TX, GY * GX, DH], f32,
                           kind="ExternalOutput").ap()

    with tile.TileContext(nc) as tc:
        with ExitStack() as ctx:
            cpool = ctx.enter_context(tc.tile_pool(name="const", bufs=1))
            spool = ctx.enter_context(tc.tile_pool(name="sb", bufs=2))
            vpool = ctx.enter_context(tc.tile_pool(name="vt", bufs=6))
            opool = ctx.enter_context(tc.tile_pool(name="osb", bufs=3))
            zpool = ctx.enter_context(tc.tile_pool(name="rz", bufs=3))
            pp_s = ctx.enter_context(
                tc.tile_pool(name="ps_scores", bufs=2, space="PSUM"))
            pp_o = ctx.enter_context(
                tc.tile_pool(name="ps_out", bufs=2, space="PSUM"))

            NT = GY * GX
            q_sb = cpool.tile([D, NT, TY * TX], bf16)
            k_sb = cpool.tile([D, NT, HY * HX], bf16)
            vt_sb = cpool.tile([110, NT, 3, DH + 1], bf16)
            out_all = cpool.tile([TY * TX, NT, DH], f32)
            # sliced loads so early tiles unblock before the full load lands
            GRP = 12
            for g in range(NT // GRP):
                s = slice(GRP * g, GRP * (g + 1))
                nc.sync.dma_start(q_sb[:, s, :], q_d[:, s, :])
                nc.sync.dma_start(k_sb[:, s, :], k_d[:, s, :])
                nc.sync.dma_start(vt_sb[:, s, :, :], vt_d[:, s, :, :])

            for pair in range(NT // 2):
                scores = pp_s.tile([128, 6, 128], f32, tag="scores")
                for half in range(2):
                    t = 2 * pair + half
                    for c, (r0, nr) in enumerate(CHUNKS):
                        nk = nr * HX
                        nc.tensor.matmul(
                            scores[:nk, 3 * half + c, :],
                            lhsT=k_sb[:, t, HX * r0:HX * r0 + nk],
                            rhs=q_sb[:, t, :],
                            start=True, stop=True)
                expT = spool.tile([128, 6, 128], bf16, tag="expT")
                nc.scalar.activation(expT[:], scores[:],
                                     mybir.ActivationFunctionType.Exp,
                                     scale=0.25)
                for half in range(2):
                    t = 2 * pair + half
                    out_u = pp_o.tile([128, DH + 1], f32, tag="outu")
                    for c, (r0, nr) in enumerate(CHUNKS):
                        nk = nr * HX
                        nc.tensor.matmul(
                            out_u[:],
                            lhsT=expT[:nk, 3 * half + c, :],
                            rhs=vt_sb[:nk, t, c, :],
                            start=(c == 0), stop=(c == 2))
                    rz = zpool.tile([128, 1], f32, tag="rz")
                    nc.vector.reciprocal(rz[:], out_u[:, DH:DH + 1])
                    nc.vector.tensor_scalar_mul(out_all[:, t, :],
                                                out_u[:, :DH], rz[:])
            for g in range(NT // GRP):
                s = slice(GRP * g, GRP * (g + 1))
                nc.scalar.dma_start(out_d[:, s, :], out_all[:, s, :])
    nc.compile()
    return nc


def kernel(q, k, v):
    from concourse.bass_utils import run_bass_kernel_spmd

    nc = _build_program()
    in_maps = _host_prep(np.asarray(q, np.float32), np.asarray(k, np.float32),
                         np.asarray(v, np.float32))
    res = run_bass_kernel_spmd(nc, in_maps, list(range(8)))

    out = np.empty((B, HEADS, DH, H, W), np.float32)
    for core in range(8):
        b, h = divmod(core, HEADS)
        # [TY*TX, NT, DH] -> [qy,qx,ty,tx,d] -> [ty,qy,tx,qx,d] -> [H,W,DH]
        o = res.results[core]["out"].reshape(TY, TX, GY, GX, DH)
        o = o.transpose(2, 0, 3, 1, 4).reshape(H, W, DH)
        out[b, h] = np.transpose(o, (2, 0, 1))
    return out.reshape(B, HEADS * DH, H, W)

